# revision 23
# baseline (speedup 1.0000x reference)
"""MoE top-2/8 SwiGLU Trainium2 Bass kernel.

Sharding: data-parallel over tokens — the 8192 tokens (B*S) are split into
8 slices of 1024, one per NeuronCore; expert weights are replicated.

Per core:
  1. Router: logits via fp32 matmuls (full precision so top-2 selection
     never flips vs the reference), softmax, top-2 + renormalized weights.
  2. Slot positions: per-(token,expert) rank among the expert's tokens via
     triangular/ones matmul cumsum over the top-2 mask.
  3. Inverse permutation: indicator I[t,s] = (pos*mask == s+1) contracted
     with [token-id | weights] gives each expert slot's token id and weight
     (CAP=384 slots/expert; observed max count ~290 across backends).
  4. Per expert: indirect-DMA gather of its token rows (bf16), PE-transpose
     to (H, slots), GEMM1 (bf16) + SwiGLU, GEMM2 (bf16), scale rows by the
     routing weight, store to a compact DRAM y-slot buffer (bf16).
  5. Final: each token indirect-gathers its two slot rows, adds, writes out.
"""

import numpy as np
import ml_dtypes

import concourse.bass as bass
import concourse.bacc as bacc
import concourse.mybir as mybir
import concourse.tile as tile
from concourse.bass_utils import run_bass_kernel_spmd
from concourse.masks import make_upper_triangular, make_identity

F32 = mybir.dt.float32
F32R = mybir.dt.float32r
BF16 = mybir.dt.bfloat16
I32 = mybir.dt.int32

E, H, I2, I = 8, 1024, 4096, 2048
NCORES = 8
T = 1024
P = 128
KT = H // P          # 8
CAP = 280            # slots per expert (host rebalancing keeps counts <= 271)
SZ = [128, 128, 24]  # slot tile sizes
SOFF = [0, 128, 256]
ST = len(SZ)
NT = T // P          # 8
BIG = 32768.0

Copy = mybir.ActivationFunctionType.Copy
Exp = mybir.ActivationFunctionType.Exp
Silu = mybir.ActivationFunctionType.Silu
Alu = mybir.AluOpType

LAST_RESULTS = None


def _build_program():
    nc = bacc.Bacc(None)
    xT = nc.declare_dram_parameter("xT", [H, T], F32, isOutput=False)
    xrow = nc.declare_dram_parameter("xrow", [T, H], BF16, isOutput=False)
    rwT = nc.declare_dram_parameter("rwT", [H, E], F32, isOutput=False)
    w13 = nc.declare_dram_parameter("w13", [E, H, I2], BF16, isOutput=False)
    # w2 host-reblocked: w2c[e, h, p, kk2, c] = w2[e, kk2*128+p, h*128+c]
    w2c = nc.declare_dram_parameter("w2c", [E, H // P, P, I // P, P], BF16,
                                    isOutput=False)
    out = nc.declare_dram_parameter("out", [T, H], BF16, isOutput=True)
    # yab rows: 0 unused, 1..T top-1 contributions (token order), T+1..2T
    # top-2 contributions, 2T+1+e a per-expert trash row for padding slots
    yab = nc.dram_tensor("yab", [2 * T + 1 + E, H], BF16)

    with tile.TileContext(nc) as tc:
        with tc.tile_pool(name="persist", bufs=1) as pp, \
             tc.tile_pool(name="w13p", bufs=3) as wp1, \
             tc.tile_pool(name="w2p", bufs=3) as wp2, \
             tc.tile_pool(name="hp", bufs=1) as hp, \
             tc.tile_pool(name="xgp", bufs=4) as xgp, \
             tc.tile_pool(name="eqp", bufs=2) as eqp, \
             tc.tile_pool(name="xtp", bufs=3) as xtp, \
             tc.tile_pool(name="yp", bufs=3) as yp, \
             tc.tile_pool(name="tmp", bufs=4) as tp, \
             tc.tile_pool(name="ps1", bufs=3, space="PSUM") as ps1, \
             tc.tile_pool(name="ps2", bufs=2, space="PSUM") as ps2, \
             tc.tile_pool(name="psp", bufs=1, space="PSUM") as psp, \
             tc.tile_pool(name="ptr", bufs=2, space="PSUM") as ptr:

            # ---------------- constants ----------------
            ident32 = pp.tile([P, P], F32, tag="ident32")
            make_identity(nc, ident32[:])
            identb = pp.tile([P, P], BF16, tag="identb")
            nc.vector.tensor_copy(out=identb[:], in_=ident32[:])
            tri32 = pp.tile([P, P], F32, tag="tri32")
            make_upper_triangular(nc, tri32[:], val=1.0, diag=True)
            trir = pp.tile([P, P], F32R, tag="trir")
            nc.vector.tensor_copy(out=trir[:], in_=tri32[:])
            ones32 = pp.tile([P, P], F32, tag="ones32")
            nc.vector.memset(ones32[:], 1.0)
            onesr = pp.tile([P, P], F32R, tag="onesr")
            nc.vector.tensor_copy(out=onesr[:], in_=ones32[:])

            iotai = pp.tile([P, CAP], I32, tag="iotai")
            nc.gpsimd.iota(iotai[:], pattern=[[1, CAP]], base=1,
                           channel_multiplier=0)
            iotaf = pp.tile([P, CAP], F32, tag="iotaf")
            nc.vector.tensor_copy(out=iotaf[:], in_=iotai[:])

            repc = pp.tile([P, 8], F32, tag="repc")
            nc.vector.memset(repc[:], -1.0)
            toki = pp.tile([P, NT], I32, tag="toki")
            nc.gpsimd.iota(toki[:], pattern=[[P, NT]], base=0,
                           channel_multiplier=1)   # toki[p, m] = m*128 + p
            tokr = pp.tile([P, NT], F32R, tag="tokr")
            nc.vector.tensor_copy(out=tokr[:], in_=toki[:])
            tokf = pp.tile([P, NT], F32, tag="tokf")
            nc.vector.tensor_copy(out=tokf[:], in_=toki[:])
            # cc[:, e] = 1 + T - Ke where Ke = 2T+1+e is expert e's trash row;
            # dest'[t, e] = tok + cc[e] - T*is_top1 so that pips pad slots
            # (which sum to zero) land on the trash row after adding Ke back.
            cc = pp.tile([P, E], F32, tag="cc")
            for e in range(E):
                nc.vector.memset(cc[:, e:e + 1], float(1 + T - (2 * T + 1 + e)))

            # ---------------- load xT, router weights ----------------
            rwt = pp.tile([P, KT, E], F32, tag="rwt")
            nc.sync.dma_start(
                out=rwt[:], in_=rwT.rearrange("(kk p) e -> p kk e", p=P))
            xt = [pp.tile([P, T], F32, tag=f"xt{kk}", name=f"xtt{kk}")
                  for kk in range(KT)]
            for m in range(2):
                for kk in range(KT):
                    nc.sync.dma_start(
                        out=xt[kk][:, m * P:(m + 1) * P],
                        in_=xT[kk * P:(kk + 1) * P, m * P:(m + 1) * P])
            for kk in range(KT):
                nc.sync.dma_start(out=xt[kk][:, 2 * P:T],
                                  in_=xT[kk * P:(kk + 1) * P, 2 * P:T])

            # ---------------- router + slot positions ----------------
            # riw columns: 0 token id, 1..8 combine weight per expert,
            # 9..16 scatter destination row per expert (offset by -Ke)
            NW = 1 + E + E
            # expert-0 inverse-perm accumulates inside the router loop so its
            # gather can fire as soon as routing finishes
            pipsT0 = psp.tile([NW, CAP], F32, tag="pips", name="pip0")
            maskr, qtiles, rhsiw = [], [], []
            for m in range(NT):
                pl = ps2.tile([P, E], F32, tag="ps2", name=f"pl{m}")
                for kk in range(KT):
                    nc.tensor.matmul(
                        pl[:], xt[kk][:, m * P:(m + 1) * P], rwt[:, kk, :],
                        start=(kk == 0), stop=(kk == KT - 1))
                top8l = tp.tile([P, 8], F32, tag="t8l", name="t8l")
                nc.vector.max(out=top8l[:], in_=pl[:])
                negm = tp.tile([P, 1], F32, tag="negm", name="negm")
                nc.vector.tensor_scalar_mul(negm[:], top8l[:, 0:1], -1.0)
                exps = tp.tile([P, E], F32, tag="exps", name="exps")
                sume = tp.tile([P, 1], F32, tag="sume", name="sume")
                nc.scalar.activation(out=exps[:], in_=pl[:], func=Exp,
                                     bias=negm[:, 0:1], accum_out=sume[:, 0:1])
                rz = tp.tile([P, 1], F32, tag="rz", name="rz")
                nc.vector.reciprocal(rz[:], sume[:])
                probs = tp.tile([P, E], F32, tag="probs", name="probs")
                nc.vector.tensor_scalar_mul(probs[:], exps[:], rz[:, 0:1])
                top8p = tp.tile([P, 8], F32, tag="t8p", name="t8p")
                nc.vector.max(out=top8p[:], in_=probs[:])
                den = tp.tile([P, 1], F32, tag="den", name="den")
                nc.vector.tensor_scalar(den[:], top8p[:, 0:1],
                                        top8p[:, 1:2], 1e-6,
                                        Alu.add, Alu.add)
                rden = tp.tile([P, 1], F32, tag="rden", name="rden")
                nc.vector.reciprocal(rden[:], den[:])
                repin = tp.tile([P, 8], F32, tag="repin", name="repin")
                nc.vector.tensor_copy(out=repin[:, 2:8], in_=repc[:, 2:8])
                nc.vector.tensor_copy(out=repin[:, 0:2], in_=top8p[:, 0:2])
                repl = tp.tile([P, 8], F32, tag="repl", name="repl")
                nc.vector.match_replace(out=repl[:], in_to_replace=repin[:],
                                        in_values=probs[:], imm_value=-2.0)
                mask = tp.tile([P, E], F32, tag="maskt", name="maskt")
                nc.vector.tensor_tensor(out=mask[:], in0=probs[:], in1=repl[:],
                                        op=Alu.not_equal)
                mr = pp.tile([P, E], F32R, tag=f"maskr{m}", name=f"maskr{m}")
                nc.vector.tensor_copy(out=mr[:], in_=mask[:])
                maskr.append(mr)
                cw = tp.tile([P, E], F32, tag="cw", name="cw")
                nc.vector.tensor_tensor(out=cw[:], in0=probs[:], in1=mask[:],
                                        op=Alu.mult)
                nc.vector.tensor_scalar_mul(cw[:], cw[:], rden[:, 0:1])

                ppos = ps2.tile([P, E], F32, tag="ps2", name=f"ppos{m}")
                if m == 0:
                    nc.tensor.matmul(ppos[:], trir[:], maskr[0][:],
                                     start=True, stop=True)
                else:
                    for mp in range(m):
                        nc.tensor.matmul(ppos[:], onesr[:], maskr[mp][:],
                                         start=(mp == 0), stop=False)
                    nc.tensor.matmul(ppos[:], trir[:], maskr[m][:],
                                     start=False, stop=True)
                q = pp.tile([P, E], F32, tag=f"q{m}", name=f"q{m}")
                nc.vector.tensor_tensor(out=q[:], in0=ppos[:], in1=mask[:],
                                        op=Alu.mult)
                qtiles.append(q)

                # scatter destination rows: dest'[t, e] = tok + cc[e]
                # - T*is_top1 (pips pad slots sum to 0 -> trash row via +Ke)
                ist1 = tp.tile([P, E], F32, tag="ist1", name="ist1")
                nc.vector.tensor_tensor(
                    out=ist1[:], in0=probs[:],
                    in1=top8p[:, 0:1].to_broadcast([P, E]), op=Alu.is_equal)
                dst = tp.tile([P, E], F32, tag="dst", name="dst")
                nc.vector.tensor_scalar_mul(dst[:], ist1[:], -float(T))
                nc.vector.tensor_tensor(out=dst[:], in0=dst[:], in1=cc[:],
                                        op=Alu.add)
                nc.vector.tensor_tensor(
                    out=dst[:], in0=dst[:],
                    in1=tokf[:, m:m + 1].to_broadcast([P, E]), op=Alu.add)

                riw = pp.tile([P, NW], F32R, tag=f"riw{m}", name=f"riw{m}")
                nc.vector.tensor_copy(out=riw[:, 0:1], in_=tokr[:, m:m + 1])
                nc.vector.tensor_copy(out=riw[:, 1:1 + E], in_=cw[:])
                nc.vector.tensor_copy(out=riw[:, 1 + E:NW], in_=dst[:])
                rhsiw.append(riw)

                it0 = tp.tile([P, CAP], F32R, tag="ieq0", name="ieq0")
                nc.vector.tensor_tensor(
                    out=it0[:],
                    in0=q[:, 0:1].to_broadcast([P, CAP]),
                    in1=iotaf[:], op=Alu.is_equal)
                nc.tensor.matmul(pipsT0[:], riw[:], it0[:],
                                 start=(m == 0), stop=(m == NT - 1))

            # ---------------- inverse permutation per expert ----------------
            sidx = [[None] * ST for _ in range(E)]
            swt = [[None] * ST for _ in range(E)]
            sdst = [[None] * ST for _ in range(E)]

            def perm_extract(e, pipsT):
                # pipsT is [NW, CAP] psum; transpose each slot tile back to
                # (slots, NW) and pull out index / weight / dest columns.
                pts = tp.tile([NW, CAP], F32, tag="ptsb", name=f"ptsb{e}")
                nc.vector.tensor_copy(out=pts[:], in_=pipsT[:])
                for st in range(ST):
                    sz = SZ[st]
                    tps = ptr.tile([P, NW], F32, tag="ptr",
                                   name=f"tps{e}_{st}")
                    nc.tensor.transpose(
                        out=tps[:sz, :NW],
                        in_=pts[:, SOFF[st]:SOFF[st] + sz],
                        identity=ident32[:NW, :NW])
                    si = pp.tile([SZ[st], 1], I32, tag=f"si{e}_{st}",
                                 name=f"si{e}_{st}")
                    nc.vector.tensor_copy(out=si[:], in_=tps[:sz, 0:1])
                    sw = pp.tile([SZ[st], 1], F32, tag=f"sw{e}_{st}",
                                 name=f"sw{e}_{st}")
                    nc.vector.tensor_copy(out=sw[:],
                                          in_=tps[:sz, 1 + e:2 + e])
                    sd = pp.tile([SZ[st], 1], I32, tag=f"sd{e}_{st}",
                                 name=f"sd{e}_{st}")
                    nc.vector.tensor_scalar(
                        sd[:], tps[:sz, 1 + E + e:2 + E + e],
                        float(2 * T + 1 + e), 0.0, Alu.add, Alu.add)
                    sidx[e][st] = si
                    swt[e][st] = sw
                    sdst[e][st] = sd

            perm_extract(0, pipsT0)

            def perm_eq(e):
                its = []
                for m in range(NT):
                    it = eqp.tile([P, CAP], F32R, tag=f"ieq{m}",
                                  name=f"ieq{e}_{m}")
                    nc.vector.tensor_tensor(
                        out=it[:],
                        in0=qtiles[m][:, e:e + 1].to_broadcast([P, CAP]),
                        in1=iotaf[:],
                        op=Alu.is_equal)
                    its.append(it)
                return its

            def perm_block(e, its):
                pipsT = psp.tile([NW, CAP], F32, tag="pips", name=f"pip{e}")
                for m in range(NT):
                    nc.tensor.matmul(pipsT[:], rhsiw[m][:], its[m][:],
                                     start=(m == 0), stop=(m == NT - 1))
                perm_extract(e, pipsT)

            # ---------------- per-expert compute (sw-pipelined) ----------
            hsb = [None] * 16

            def gather_and_transpose(e):
                xgt = [xtp.tile([P, CAP], BF16, tag=f"xgt{kk}",
                                name=f"xgt{kk}_{e}") for kk in range(KT)]
                for st in range(ST):
                    sz = SZ[st]
                    xg = xgp.tile([P, H], BF16, tag="xg", name=f"xg{e}_{st}")
                    nc.gpsimd.indirect_dma_start(
                        out=xg[:sz, :], out_offset=None,
                        in_=xrow[:],
                        in_offset=bass.IndirectOffsetOnAxis(
                            ap=sidx[e][st][:, 0:1], axis=0))
                    for kk in range(KT):
                        pt = ptr.tile([P, P], BF16, tag="ptr",
                                      name=f"pt{e}_{st}_{kk}")
                        nc.tensor.transpose(
                            out=pt[:P, :sz], in_=xg[:sz, kk * P:(kk + 1) * P],
                            identity=identb[:sz, :sz])
                        nc.vector.tensor_copy(
                            out=xgt[kk][:, SOFF[st]:SOFF[st] + sz],
                            in_=pt[:P, :sz])
                return xgt

            xgt_next = gather_and_transpose(0)
            its_next = perm_eq(1)
            for e in range(E):
                xgt = xgt_next

                # GEMM1 (bf16) + SwiGLU -> h (bf16), transposed (I, slots)
                w13r = w13[e].rearrange("(kk p) i -> p kk i", p=P)
                for c in range(8):
                    wt = wp1.tile([P, KT, 512], BF16, tag="w13t",
                                  name=f"w13t{e}_{c}")
                    nc.sync.dma_start(
                        out=wt[:], in_=w13r[:, :, c * 512:(c + 1) * 512])
                    for j in range(4):
                        g = c * 4 + j
                        pg = ps1.tile([P, CAP], F32, tag="ps1",
                                      name=f"pg{e}_{g}")
                        for kk in range(KT):
                            nc.tensor.matmul(
                                pg[:], wt[:, kk, j * P:(j + 1) * P],
                                xgt[kk][:],
                                start=(kk == 0), stop=(kk == KT - 1))
                        if g < 16:
                            ht = hp.tile([P, CAP], BF16, tag=f"h{g}",
                                         name=f"h{g}_{e}")
                            hsb[g] = ht
                            nc.scalar.activation(out=ht[:], in_=pg[:],
                                                 func=Silu)
                        else:
                            nc.vector.tensor_tensor(
                                out=hsb[g - 16][:], in0=hsb[g - 16][:],
                                in1=pg[:], op=Alu.mult)

                if e + 1 < E:
                    perm_block(e + 1, its_next)
                    xgt_next = gather_and_transpose(e + 1)
                    if e + 2 < E:
                        its_next = perm_eq(e + 2)

                # GEMM2 (bf16): yT[h-tile, slots] = w2[e]^T-blocks @ h, so the
                # matmul free dim is the slot count; transpose back per slot
                # tile with per-slot routing-weight scaling on the psum read.
                ysb = [yp.tile([SZ[st], H], BF16, tag=f"ysb{st}",
                               name=f"ysb{e}_{st}") for st in range(ST)]
                for h in range(KT):
                    w2ct = wp2.tile([P, I // P, P], BF16, tag="w2t",
                                    name=f"w2t{e}_{h}")
                    nc.sync.dma_start(out=w2ct[:], in_=w2c[e, h])
                    pyt = ps2.tile([P, CAP], F32, tag="ps2",
                                   name=f"pyt{e}_{h}")
                    for kk2 in range(16):
                        nc.tensor.matmul(pyt[:], w2ct[:, kk2, :],
                                         hsb[kk2][:],
                                         start=(kk2 == 0), stop=(kk2 == 15))
                    ytb = tp.tile([P, CAP], BF16, tag="ytb",
                                  name=f"ytb{e}_{h}")
                    nc.vector.tensor_copy(out=ytb[:], in_=pyt[:])
                    for st in range(ST):
                        sz = SZ[st]
                        ptt = ptr.tile([P, P], BF16, tag="ptr",
                                       name=f"yt{e}_{h}_{st}")
                        nc.tensor.transpose(
                            out=ptt[:sz, :P],
                            in_=ytb[:, SOFF[st]:SOFF[st] + sz],
                            identity=identb[:, :])
                        nc.scalar.activation(
                            out=ysb[st][:, h * P:(h + 1) * P],
                            in_=ptt[:sz, :P], func=Copy,
                            scale=swt[e][st][:, 0:1])
                for st in range(ST):
                    nc.gpsimd.indirect_dma_start(
                        out=yab[:],
                        out_offset=bass.IndirectOffsetOnAxis(
                            ap=sdst[e][st][:, 0:1], axis=0),
                        in_=ysb[st][:], in_offset=None)

            # ---------------- final combine ----------------
            # yab rows 1..T and T+1..2T hold each token's two scaled expert
            # outputs in token order; combine is two contiguous streams + add.
            for m in range(NT):
                ga = tp.tile([P, H], BF16, tag="ga", name=f"ga{m}")
                nc.gpsimd.dma_start(
                    out=ga[:], in_=yab[1 + m * P:1 + (m + 1) * P, :])
                gb = tp.tile([P, H], BF16, tag="gb", name=f"gb{m}")
                nc.sync.dma_start(
                    out=gb[:], in_=yab[1 + T + m * P:1 + T + (m + 1) * P, :])
                go = tp.tile([P, H], BF16, tag="go", name=f"go{m}")
                nc.vector.tensor_tensor(out=go[:], in0=ga[:], in1=gb[:],
                                        op=Alu.add)
                nc.sync.dma_start(out=out[m * P:(m + 1) * P, :], in_=go[:])

    nc.compile()
    return nc


_prog = None


def _balanced_token_perm(xrows, router_w):
    """Assign tokens to cores so per-(core, expert) routed counts stay
    well under CAP (global max expert load / 8 is ~271).  Routing here is
    the same fp32 math the device performs; the min top2/top3 probability
    gap in this data (~2e-5) is far above fp32 noise, so host and device
    agree on the selected experts."""
    logits = (xrows @ router_w.T).astype(np.float32)
    m = logits.max(-1, keepdims=True)
    p = np.exp(logits - m)
    p /= p.sum(-1, keepdims=True)
    idx = np.argsort(-p, axis=-1)[:, :2]
    N = xrows.shape[0]
    counts = np.zeros((NCORES, E), dtype=np.int64)
    sizes = np.zeros(NCORES, dtype=np.int64)
    asgn = np.empty(N, dtype=np.int64)
    for t in range(N):
        e1, e2 = idx[t]
        best, bkey = -1, None
        for c in range(NCORES):
            if sizes[c] >= T:
                continue
            key = (max(counts[c, e1], counts[c, e2]),
                   counts[c, e1] + counts[c, e2], sizes[c])
            if bkey is None or key < bkey:
                bkey, best = key, c
        asgn[t] = best
        counts[best, e1] += 1
        counts[best, e2] += 1
        sizes[best] += 1
    assert counts.max() <= CAP - 4, f"capacity overflow risk: {counts.max()}"
    return np.argsort(asgn, kind="stable")


def kernel(x, router_w, w13, w2):
    global _prog, LAST_RESULTS
    if _prog is None:
        _prog = _build_program()
    nc = _prog

    xrows = x.reshape(NCORES * T, H).astype(np.float32)
    perm = _balanced_token_perm(xrows, np.asarray(router_w, np.float32))
    xrows = np.ascontiguousarray(xrows[perm])
    xt_full = np.ascontiguousarray(xrows.T)
    rwT_np = np.ascontiguousarray(router_w.T).astype(np.float32)
    w13_b = np.ascontiguousarray(w13).astype(ml_dtypes.bfloat16)
    # w2c[e, h, p, kk2, c] = w2[e, kk2*128+p, h*128+c]
    w2_b = np.ascontiguousarray(
        np.asarray(w2).reshape(E, I // 128, 128, H // 128, 128)
        .transpose(0, 3, 2, 1, 4)).astype(ml_dtypes.bfloat16)

    in_maps = []
    for c in range(NCORES):
        in_maps.append({
            "xT": np.ascontiguousarray(xt_full[:, c * T:(c + 1) * T]),
            "xrow": np.ascontiguousarray(
                xrows[c * T:(c + 1) * T]).astype(ml_dtypes.bfloat16),
            "rwT": rwT_np,
            "w13": w13_b,
            "w2c": w2_b,
        })

    res = run_bass_kernel_spmd(nc, in_maps, core_ids=list(range(NCORES)))
    LAST_RESULTS = res
    outs = [np.asarray(res.results[c]["out"]).astype(np.float32)
            for c in range(NCORES)]
    full = np.concatenate(outs, axis=0)
    unperm = np.empty_like(full)
    unperm[perm] = full
    return unperm.reshape(4, 2048, H).astype(x.dtype, copy=False)



# revision 29
# speedup vs baseline: 1.4863x; 1.4863x over previous
"""MoE top-2/8 SwiGLU Trainium2 Bass kernel.

Sharding: data-parallel over tokens — the 8192 tokens (B*S) are split into
8 slices of 1024, one per NeuronCore; expert weights are replicated.

Per core:
  1. Router: logits via fp32 matmuls (full precision so top-2 selection
     never flips vs the reference), softmax, top-2 + renormalized weights.
  2. Slot positions: per-(token,expert) rank among the expert's tokens via
     triangular/ones matmul cumsum over the top-2 mask.
  3. Inverse permutation: indicator I[t,s] = (pos*mask == s+1) contracted
     with [token-id | weights] gives each expert slot's token id and weight
     (CAP=384 slots/expert; observed max count ~290 across backends).
  4. Per expert: indirect-DMA gather of its token rows (bf16), PE-transpose
     to (H, slots), GEMM1 (bf16) + SwiGLU, GEMM2 (bf16), scale rows by the
     routing weight, store to a compact DRAM y-slot buffer (bf16).
  5. Final: each token indirect-gathers its two slot rows, adds, writes out.
"""

import numpy as np
import ml_dtypes

import concourse.bass as bass
import concourse.bacc as bacc
import concourse.mybir as mybir
import concourse.tile as tile
from concourse.bass_utils import run_bass_kernel_spmd
from concourse.masks import make_upper_triangular, make_identity

F32 = mybir.dt.float32
F32R = mybir.dt.float32r
BF16 = mybir.dt.bfloat16
I32 = mybir.dt.int32

E, H, I2, I = 8, 1024, 4096, 2048
NCORES = 8
T = 1024
P = 128
KT = H // P          # 8
CAP = 280            # slots per expert (host rebalancing keeps counts <= 271)
SZ = [128, 128, 24]  # slot tile sizes
SOFF = [0, 128, 256]
ST = len(SZ)
NT = T // P          # 8
BIG = 32768.0

Copy = mybir.ActivationFunctionType.Copy
Exp = mybir.ActivationFunctionType.Exp
Silu = mybir.ActivationFunctionType.Silu
Alu = mybir.AluOpType

LAST_RESULTS = None


def _build_program():
    nc = bacc.Bacc(None)
    xT = nc.declare_dram_parameter("xT", [H, T], F32, isOutput=False)
    xrow = nc.declare_dram_parameter("xrow", [T, H], BF16, isOutput=False)
    rwT = nc.declare_dram_parameter("rwT", [H, E], F32, isOutput=False)
    w13 = nc.declare_dram_parameter("w13", [E, H, I2], BF16, isOutput=False)
    # w2 host-reblocked: w2c[e, h, p, kk2, c] = w2[e, kk2*128+p, h*128+c]
    w2c = nc.declare_dram_parameter("w2c", [E, H // P, P, I // P, P], BF16,
                                    isOutput=False)
    out = nc.declare_dram_parameter("out", [T, H], BF16, isOutput=True)
    yslots = nc.dram_tensor("yslots", [E * CAP, H], BF16)

    with tile.TileContext(nc) as tc:
        with tc.tile_pool(name="persist", bufs=1) as pp, \
             tc.tile_pool(name="w13p", bufs=3) as wp1, \
             tc.tile_pool(name="w2p", bufs=3) as wp2, \
             tc.tile_pool(name="hp", bufs=1) as hp, \
             tc.tile_pool(name="xgp", bufs=4) as xgp, \
             tc.tile_pool(name="eqp", bufs=2) as eqp, \
             tc.tile_pool(name="xtp", bufs=3) as xtp, \
             tc.tile_pool(name="yp", bufs=3) as yp, \
             tc.tile_pool(name="tmp", bufs=4) as tp, \
             tc.tile_pool(name="ps1", bufs=3, space="PSUM") as ps1, \
             tc.tile_pool(name="ps2", bufs=2, space="PSUM") as ps2, \
             tc.tile_pool(name="psp", bufs=1, space="PSUM") as psp, \
             tc.tile_pool(name="ptr", bufs=2, space="PSUM") as ptr:

            # ---------------- constants ----------------
            ident32 = pp.tile([P, P], F32, tag="ident32")
            make_identity(nc, ident32[:])
            identb = pp.tile([P, P], BF16, tag="identb")
            nc.vector.tensor_copy(out=identb[:], in_=ident32[:])
            tri32 = pp.tile([P, P], F32, tag="tri32")
            make_upper_triangular(nc, tri32[:], val=1.0, diag=True)
            trir = pp.tile([P, P], F32R, tag="trir")
            nc.vector.tensor_copy(out=trir[:], in_=tri32[:])
            ones32 = pp.tile([P, P], F32, tag="ones32")
            nc.vector.memset(ones32[:], 1.0)
            onesr = pp.tile([P, P], F32R, tag="onesr")
            nc.vector.tensor_copy(out=onesr[:], in_=ones32[:])

            iotai = pp.tile([P, CAP], I32, tag="iotai")
            nc.gpsimd.iota(iotai[:], pattern=[[1, CAP]], base=1,
                           channel_multiplier=0)
            iotaf = pp.tile([P, CAP], F32, tag="iotaf")
            nc.vector.tensor_copy(out=iotaf[:], in_=iotai[:])

            repc = pp.tile([P, 8], F32, tag="repc")
            nc.vector.memset(repc[:], -1.0)
            toki = pp.tile([P, NT], I32, tag="toki")
            nc.gpsimd.iota(toki[:], pattern=[[P, NT]], base=0,
                           channel_multiplier=1)   # toki[p, m] = m*128 + p
            tokr = pp.tile([P, NT], F32R, tag="tokr")
            nc.vector.tensor_copy(out=tokr[:], in_=toki[:])
            ebase = pp.tile([P, E], F32, tag="ebase")
            for e in range(E):
                nc.vector.memset(ebase[:, e:e + 1], float(e * CAP))

            # ---------------- load xT, router weights ----------------
            rwt = pp.tile([P, KT, E], F32, tag="rwt")
            nc.sync.dma_start(
                out=rwt[:], in_=rwT.rearrange("(kk p) e -> p kk e", p=P))
            xt = [pp.tile([P, T], F32, tag=f"xt{kk}", name=f"xtt{kk}")
                  for kk in range(KT)]
            for m in range(2):
                for kk in range(KT):
                    nc.sync.dma_start(
                        out=xt[kk][:, m * P:(m + 1) * P],
                        in_=xT[kk * P:(kk + 1) * P, m * P:(m + 1) * P])
            for kk in range(KT):
                nc.sync.dma_start(out=xt[kk][:, 2 * P:T],
                                  in_=xT[kk * P:(kk + 1) * P, 2 * P:T])

            # ---------------- router + slot positions ----------------
            # riw columns: 0 token id, 1..8 combine weight per expert
            NW = 1 + E
            # expert-0 inverse-perm accumulates inside the router loop so its
            # gather can fire as soon as routing finishes
            pipsT0 = psp.tile([NW, CAP], F32, tag="pips", name="pip0")
            maskr, qtiles, rhsiw, sidx_ab = [], [], [], []
            for m in range(NT):
                pl = ps2.tile([P, E], F32, tag="ps2", name=f"pl{m}")
                for kk in range(KT):
                    nc.tensor.matmul(
                        pl[:], xt[kk][:, m * P:(m + 1) * P], rwt[:, kk, :],
                        start=(kk == 0), stop=(kk == KT - 1))
                top8l = tp.tile([P, 8], F32, tag="t8l", name="t8l")
                nc.vector.max(out=top8l[:], in_=pl[:])
                negm = tp.tile([P, 1], F32, tag="negm", name="negm")
                nc.vector.tensor_scalar_mul(negm[:], top8l[:, 0:1], -1.0)
                exps = tp.tile([P, E], F32, tag="exps", name="exps")
                sume = tp.tile([P, 1], F32, tag="sume", name="sume")
                nc.scalar.activation(out=exps[:], in_=pl[:], func=Exp,
                                     bias=negm[:, 0:1], accum_out=sume[:, 0:1])
                rz = tp.tile([P, 1], F32, tag="rz", name="rz")
                nc.vector.reciprocal(rz[:], sume[:])
                probs = tp.tile([P, E], F32, tag="probs", name="probs")
                nc.vector.tensor_scalar_mul(probs[:], exps[:], rz[:, 0:1])
                top8p = tp.tile([P, 8], F32, tag="t8p", name="t8p")
                nc.vector.max(out=top8p[:], in_=probs[:])
                den = tp.tile([P, 1], F32, tag="den", name="den")
                nc.vector.tensor_scalar(den[:], top8p[:, 0:1],
                                        top8p[:, 1:2], 1e-6,
                                        Alu.add, Alu.add)
                rden = tp.tile([P, 1], F32, tag="rden", name="rden")
                nc.vector.reciprocal(rden[:], den[:])
                repin = tp.tile([P, 8], F32, tag="repin", name="repin")
                nc.vector.tensor_copy(out=repin[:, 2:8], in_=repc[:, 2:8])
                nc.vector.tensor_copy(out=repin[:, 0:2], in_=top8p[:, 0:2])
                repl = tp.tile([P, 8], F32, tag="repl", name="repl")
                nc.vector.match_replace(out=repl[:], in_to_replace=repin[:],
                                        in_values=probs[:], imm_value=-2.0)
                mask = tp.tile([P, E], F32, tag="maskt", name="maskt")
                nc.vector.tensor_tensor(out=mask[:], in0=probs[:], in1=repl[:],
                                        op=Alu.not_equal)
                mr = pp.tile([P, E], F32R, tag=f"maskr{m}", name=f"maskr{m}")
                nc.vector.tensor_copy(out=mr[:], in_=mask[:])
                maskr.append(mr)
                cw = tp.tile([P, E], F32, tag="cw", name="cw")
                nc.vector.tensor_tensor(out=cw[:], in0=probs[:], in1=mask[:],
                                        op=Alu.mult)
                nc.vector.tensor_scalar_mul(cw[:], cw[:], rden[:, 0:1])

                ppos = ps2.tile([P, E], F32, tag="ps2", name=f"ppos{m}")
                if m == 0:
                    nc.tensor.matmul(ppos[:], trir[:], maskr[0][:],
                                     start=True, stop=True)
                else:
                    for mp in range(m):
                        nc.tensor.matmul(ppos[:], onesr[:], maskr[mp][:],
                                         start=(mp == 0), stop=False)
                    nc.tensor.matmul(ppos[:], trir[:], maskr[m][:],
                                     start=False, stop=True)
                q = pp.tile([P, E], F32, tag=f"q{m}", name=f"q{m}")
                nc.vector.tensor_tensor(out=q[:], in0=ppos[:], in1=mask[:],
                                        op=Alu.mult)
                qtiles.append(q)

                riw = pp.tile([P, NW], F32R, tag=f"riw{m}", name=f"riw{m}")
                nc.vector.tensor_copy(out=riw[:, 0:1], in_=tokr[:, m:m + 1])
                nc.vector.tensor_copy(out=riw[:, 1:1 + E], in_=cw[:])
                rhsiw.append(riw)

                it0 = tp.tile([P, CAP], F32R, tag="ieq0", name="ieq0")
                nc.vector.tensor_tensor(
                    out=it0[:],
                    in0=q[:, 0:1].to_broadcast([P, CAP]),
                    in1=iotaf[:], op=Alu.is_equal)
                nc.tensor.matmul(pipsT0[:], riw[:], it0[:],
                                 start=(m == 0), stop=(m == NT - 1))

                # global slot index per (t, e); BIG where not selected
                slotg = tp.tile([P, E], F32, tag="slotg", name="slotg")
                nc.vector.tensor_tensor(out=slotg[:], in0=q[:], in1=ebase[:],
                                        op=Alu.add)
                nc.vector.tensor_scalar_add(slotg[:], slotg[:], -1.0)
                slotm = tp.tile([P, E], F32, tag="slotm", name="slotm")
                nc.vector.tensor_scalar_add(slotm[:], slotg[:], -BIG)
                nc.vector.tensor_tensor(out=slotm[:], in0=slotm[:],
                                        in1=mask[:], op=Alu.mult)
                nc.vector.tensor_scalar_add(slotm[:], slotm[:], BIG)
                negs = tp.tile([P, E], F32, tag="negs", name="negs")
                nc.vector.tensor_scalar_mul(negs[:], slotm[:], -1.0)
                mn8 = tp.tile([P, 8], F32, tag="mn8", name="mn8")
                nc.vector.max(out=mn8[:], in_=negs[:])
                saf = tp.tile([P, 2], F32, tag="saf", name="saf")
                nc.vector.tensor_scalar_mul(saf[:], mn8[:, 0:2], -1.0)
                sa = pp.tile([P, 1], I32, tag=f"sa{m}", name=f"sa{m}")
                sb = pp.tile([P, 1], I32, tag=f"sb{m}", name=f"sb{m}")
                nc.vector.tensor_copy(out=sa[:], in_=saf[:, 0:1])
                nc.vector.tensor_copy(out=sb[:], in_=saf[:, 1:2])
                sidx_ab.append((sa, sb))

            # ---------------- inverse permutation per expert ----------------
            sidx = [[None] * ST for _ in range(E)]
            swt = [[None] * ST for _ in range(E)]

            def perm_extract(e, pipsT):
                # pipsT is [NW, CAP] psum; transpose each slot tile back to
                # (slots, NW) and pull out index / weight columns.
                pts = tp.tile([NW, CAP], F32, tag="ptsb", name=f"ptsb{e}")
                nc.vector.tensor_copy(out=pts[:], in_=pipsT[:])
                for st in range(ST):
                    sz = SZ[st]
                    tps = ptr.tile([P, NW], F32, tag="ptr",
                                   name=f"tps{e}_{st}")
                    nc.tensor.transpose(
                        out=tps[:sz, :NW],
                        in_=pts[:, SOFF[st]:SOFF[st] + sz],
                        identity=ident32[:NW, :NW])
                    si = pp.tile([SZ[st], 1], I32, tag=f"si{e}_{st}",
                                 name=f"si{e}_{st}")
                    nc.vector.tensor_copy(out=si[:], in_=tps[:sz, 0:1])
                    sw = pp.tile([SZ[st], 1], F32, tag=f"sw{e}_{st}",
                                 name=f"sw{e}_{st}")
                    nc.vector.tensor_copy(out=sw[:],
                                          in_=tps[:sz, 1 + e:2 + e])
                    sidx[e][st] = si
                    swt[e][st] = sw

            perm_extract(0, pipsT0)

            def perm_eq(e):
                its = []
                for m in range(NT):
                    it = eqp.tile([P, CAP], F32R, tag=f"ieq{m}",
                                  name=f"ieq{e}_{m}")
                    nc.vector.tensor_tensor(
                        out=it[:],
                        in0=qtiles[m][:, e:e + 1].to_broadcast([P, CAP]),
                        in1=iotaf[:],
                        op=Alu.is_equal)
                    its.append(it)
                return its

            def perm_block(e, its):
                pipsT = psp.tile([NW, CAP], F32, tag="pips", name=f"pip{e}")
                for m in range(NT):
                    nc.tensor.matmul(pipsT[:], rhsiw[m][:], its[m][:],
                                     start=(m == 0), stop=(m == NT - 1))
                perm_extract(e, pipsT)

            # ---------------- per-expert compute (sw-pipelined) ----------
            hsb = [None] * 16

            def gather_and_transpose(e):
                xgt = [xtp.tile([P, CAP], BF16, tag=f"xgt{kk}",
                                name=f"xgt{kk}_{e}") for kk in range(KT)]
                for st in range(ST):
                    sz = SZ[st]
                    xg = xgp.tile([P, H], BF16, tag="xg", name=f"xg{e}_{st}")
                    nc.gpsimd.indirect_dma_start(
                        out=xg[:sz, :], out_offset=None,
                        in_=xrow[:],
                        in_offset=bass.IndirectOffsetOnAxis(
                            ap=sidx[e][st][:, 0:1], axis=0))
                    for kk in range(KT):
                        pt = ptr.tile([P, P], BF16, tag="ptr",
                                      name=f"pt{e}_{st}_{kk}")
                        nc.tensor.transpose(
                            out=pt[:P, :sz], in_=xg[:sz, kk * P:(kk + 1) * P],
                            identity=identb[:sz, :sz])
                        nc.vector.tensor_copy(
                            out=xgt[kk][:, SOFF[st]:SOFF[st] + sz],
                            in_=pt[:P, :sz])
                return xgt

            xgt_next = gather_and_transpose(0)
            its_next = perm_eq(1)
            for e in range(E):
                xgt = xgt_next

                # GEMM1 (bf16) + SwiGLU -> h (bf16), transposed (I, slots)
                w13r = w13[e].rearrange("(kk p) i -> p kk i", p=P)
                for c in range(8):
                    wt = wp1.tile([P, KT, 512], BF16, tag="w13t",
                                  name=f"w13t{e}_{c}")
                    nc.sync.dma_start(
                        out=wt[:], in_=w13r[:, :, c * 512:(c + 1) * 512])
                    for j in range(4):
                        g = c * 4 + j
                        pg = ps1.tile([P, CAP], F32, tag="ps1",
                                      name=f"pg{e}_{g}")
                        for kk in range(KT):
                            nc.tensor.matmul(
                                pg[:], wt[:, kk, j * P:(j + 1) * P],
                                xgt[kk][:],
                                start=(kk == 0), stop=(kk == KT - 1))
                        if g < 16:
                            ht = hp.tile([P, CAP], BF16, tag=f"h{g}",
                                         name=f"h{g}_{e}")
                            hsb[g] = ht
                            nc.scalar.activation(out=ht[:], in_=pg[:],
                                                 func=Silu)
                        else:
                            nc.vector.tensor_tensor(
                                out=hsb[g - 16][:], in0=hsb[g - 16][:],
                                in1=pg[:], op=Alu.mult)

                if e + 1 < E:
                    perm_block(e + 1, its_next)
                    xgt_next = gather_and_transpose(e + 1)
                    if e + 2 < E:
                        its_next = perm_eq(e + 2)

                # GEMM2 (bf16): yT[h-tile, slots] = w2[e]^T-blocks @ h, so the
                # matmul free dim is the slot count; transpose back per slot
                # tile with per-slot routing-weight scaling on the psum read.
                ysb = [yp.tile([SZ[st], H], BF16, tag=f"ysb{st}",
                               name=f"ysb{e}_{st}") for st in range(ST)]
                for h in range(KT):
                    w2ct = wp2.tile([P, I // P, P], BF16, tag="w2t",
                                    name=f"w2t{e}_{h}")
                    nc.sync.dma_start(out=w2ct[:], in_=w2c[e, h])
                    pyt = ps2.tile([P, CAP], F32, tag="ps2",
                                   name=f"pyt{e}_{h}")
                    for kk2 in range(16):
                        nc.tensor.matmul(pyt[:], w2ct[:, kk2, :],
                                         hsb[kk2][:],
                                         start=(kk2 == 0), stop=(kk2 == 15))
                    ytb = tp.tile([P, CAP], BF16, tag="ytb",
                                  name=f"ytb{e}_{h}")
                    nc.vector.tensor_copy(out=ytb[:], in_=pyt[:])
                    for st in range(ST):
                        sz = SZ[st]
                        ptt = ptr.tile([P, P], BF16, tag="ptr",
                                       name=f"yt{e}_{h}_{st}")
                        nc.tensor.transpose(
                            out=ptt[:sz, :P],
                            in_=ytb[:, SOFF[st]:SOFF[st] + sz],
                            identity=identb[:, :])
                        nc.scalar.activation(
                            out=ysb[st][:, h * P:(h + 1) * P],
                            in_=ptt[:sz, :P], func=Copy,
                            scale=swt[e][st][:, 0:1])
                for st in range(ST):
                    nc.sync.dma_start(
                        out=yslots[e * CAP + SOFF[st]:
                                   e * CAP + SOFF[st] + SZ[st], :],
                        in_=ysb[st][:])

            # ---------------- final combine ----------------
            for m in range(NT):
                sa, sb = sidx_ab[m]
                ga = tp.tile([P, H], BF16, tag="ga", name=f"ga{m}")
                nc.gpsimd.indirect_dma_start(
                    out=ga[:], out_offset=None, in_=yslots[:],
                    in_offset=bass.IndirectOffsetOnAxis(ap=sa[:, 0:1], axis=0))
                gb = tp.tile([P, H], BF16, tag="gb", name=f"gb{m}")
                nc.gpsimd.indirect_dma_start(
                    out=gb[:], out_offset=None, in_=yslots[:],
                    in_offset=bass.IndirectOffsetOnAxis(ap=sb[:, 0:1], axis=0))
                go = tp.tile([P, H], BF16, tag="go", name=f"go{m}")
                nc.vector.tensor_tensor(out=go[:], in0=ga[:], in1=gb[:],
                                        op=Alu.add)
                nc.sync.dma_start(out=out[m * P:(m + 1) * P, :], in_=go[:])

    nc.compile()
    return nc


_prog = None


def _balanced_token_perm(xrows, router_w):
    """Assign tokens to cores so per-(core, expert) routed counts stay
    well under CAP (global max expert load / 8 is ~271).  Routing here is
    the same fp32 math the device performs; the min top2/top3 probability
    gap in this data (~2e-5) is far above fp32 noise, so host and device
    agree on the selected experts."""
    logits = (xrows @ router_w.T).astype(np.float32)
    m = logits.max(-1, keepdims=True)
    p = np.exp(logits - m)
    p /= p.sum(-1, keepdims=True)
    idx = np.argsort(-p, axis=-1)[:, :2]
    N = xrows.shape[0]
    counts = np.zeros((NCORES, E), dtype=np.int64)
    sizes = np.zeros(NCORES, dtype=np.int64)
    asgn = np.empty(N, dtype=np.int64)
    for t in range(N):
        e1, e2 = idx[t]
        best, bkey = -1, None
        for c in range(NCORES):
            if sizes[c] >= T:
                continue
            key = (max(counts[c, e1], counts[c, e2]),
                   counts[c, e1] + counts[c, e2], sizes[c])
            if bkey is None or key < bkey:
                bkey, best = key, c
        asgn[t] = best
        counts[best, e1] += 1
        counts[best, e2] += 1
        sizes[best] += 1
    assert counts.max() <= CAP - 4, f"capacity overflow risk: {counts.max()}"
    return np.argsort(asgn, kind="stable")


def kernel(x, router_w, w13, w2):
    global _prog, LAST_RESULTS
    if _prog is None:
        _prog = _build_program()
    nc = _prog

    xrows = x.reshape(NCORES * T, H).astype(np.float32)
    perm = _balanced_token_perm(xrows, np.asarray(router_w, np.float32))
    xrows = np.ascontiguousarray(xrows[perm])
    xt_full = np.ascontiguousarray(xrows.T)
    rwT_np = np.ascontiguousarray(router_w.T).astype(np.float32)
    w13_b = np.ascontiguousarray(w13).astype(ml_dtypes.bfloat16)
    # w2c[e, h, p, kk2, c] = w2[e, kk2*128+p, h*128+c]
    w2_b = np.ascontiguousarray(
        np.asarray(w2).reshape(E, I // 128, 128, H // 128, 128)
        .transpose(0, 3, 2, 1, 4)).astype(ml_dtypes.bfloat16)

    in_maps = []
    for c in range(NCORES):
        in_maps.append({
            "xT": np.ascontiguousarray(xt_full[:, c * T:(c + 1) * T]),
            "xrow": np.ascontiguousarray(
                xrows[c * T:(c + 1) * T]).astype(ml_dtypes.bfloat16),
            "rwT": rwT_np,
            "w13": w13_b,
            "w2c": w2_b,
        })

    res = run_bass_kernel_spmd(nc, in_maps, core_ids=list(range(NCORES)))
    LAST_RESULTS = res
    outs = [np.asarray(res.results[c]["out"]).astype(np.float32)
            for c in range(NCORES)]
    full = np.concatenate(outs, axis=0)
    unperm = np.empty_like(full)
    unperm[perm] = full
    return unperm.reshape(4, 2048, H).astype(x.dtype, copy=False)



# revision 34
# speedup vs baseline: 1.5011x; 1.0099x over previous
"""MoE top-2/8 SwiGLU Trainium2 Bass kernel.

Sharding: data-parallel over tokens — the 8192 tokens (B*S) are split into
8 slices of 1024, one per NeuronCore; expert weights are replicated.

Per core:
  1. Router: logits via fp32 matmuls (full precision so top-2 selection
     never flips vs the reference), softmax, top-2 + renormalized weights.
  2. Slot positions: per-(token,expert) rank among the expert's tokens via
     triangular/ones matmul cumsum over the top-2 mask.
  3. Inverse permutation: indicator I[t,s] = (pos*mask == s+1) contracted
     with [token-id | weights] gives each expert slot's token id and weight
     (CAP=384 slots/expert; observed max count ~290 across backends).
  4. Per expert: indirect-DMA gather of its token rows (bf16), PE-transpose
     to (H, slots), GEMM1 (bf16) + SwiGLU, GEMM2 (bf16), scale rows by the
     routing weight, store to a compact DRAM y-slot buffer (bf16).
  5. Final: each token indirect-gathers its two slot rows, adds, writes out.
"""

import numpy as np
import ml_dtypes

import concourse.bass as bass
import concourse.bacc as bacc
import concourse.mybir as mybir
import concourse.tile as tile
from concourse.bass_utils import run_bass_kernel_spmd
from concourse.masks import make_upper_triangular, make_identity

F32 = mybir.dt.float32
F32R = mybir.dt.float32r
BF16 = mybir.dt.bfloat16
I32 = mybir.dt.int32

E, H, I2, I = 8, 1024, 4096, 2048
NCORES = 8
T = 1024
P = 128
KT = H // P          # 8
CAP = 280            # slots per expert (host rebalancing keeps counts <= 271)
SZ = [128, 128, 24]  # slot tile sizes
SOFF = [0, 128, 256]
ST = len(SZ)
NT = T // P          # 8
BIG = 32768.0

Copy = mybir.ActivationFunctionType.Copy
Exp = mybir.ActivationFunctionType.Exp
Silu = mybir.ActivationFunctionType.Silu
Alu = mybir.AluOpType

LAST_RESULTS = None


def _build_program():
    nc = bacc.Bacc(None)
    xT = nc.declare_dram_parameter("xT", [H, T], F32, isOutput=False)
    xrow = nc.declare_dram_parameter("xrow", [T, H], BF16, isOutput=False)
    rwT = nc.declare_dram_parameter("rwT", [H, E], F32, isOutput=False)
    w13 = nc.declare_dram_parameter("w13", [E, H, I2], BF16, isOutput=False)
    # w2 host-reblocked: w2c[e, h, p, kk2, c] = w2[e, kk2*128+p, h*128+c]
    w2c = nc.declare_dram_parameter("w2c", [E, H // P, P, I // P, P], BF16,
                                    isOutput=False)
    out = nc.declare_dram_parameter("out", [T, H], BF16, isOutput=True)
    yslots = nc.dram_tensor("yslots", [E * CAP, H], BF16)

    with tile.TileContext(nc) as tc:
        with tc.tile_pool(name="persist", bufs=1) as pp, \
             tc.tile_pool(name="w13p", bufs=3) as wp1, \
             tc.tile_pool(name="w2p", bufs=3) as wp2, \
             tc.tile_pool(name="hp", bufs=1) as hp, \
             tc.tile_pool(name="xgp", bufs=4) as xgp, \
             tc.tile_pool(name="eqp", bufs=2) as eqp, \
             tc.tile_pool(name="xtp", bufs=3) as xtp, \
             tc.tile_pool(name="yp", bufs=3) as yp, \
             tc.tile_pool(name="tmp", bufs=4) as tp, \
             tc.tile_pool(name="ps1", bufs=3, space="PSUM") as ps1, \
             tc.tile_pool(name="ps2", bufs=2, space="PSUM") as ps2, \
             tc.tile_pool(name="psp", bufs=1, space="PSUM") as psp, \
             tc.tile_pool(name="ptr", bufs=2, space="PSUM") as ptr:

            # ---------------- constants ----------------
            ident32 = pp.tile([P, P], F32, tag="ident32")
            make_identity(nc, ident32[:])
            identb = pp.tile([P, P], BF16, tag="identb")
            nc.vector.tensor_copy(out=identb[:], in_=ident32[:])
            tri32 = pp.tile([P, P], F32, tag="tri32")
            make_upper_triangular(nc, tri32[:], val=1.0, diag=True)
            trir = pp.tile([P, P], F32R, tag="trir")
            nc.vector.tensor_copy(out=trir[:], in_=tri32[:])
            ones32 = pp.tile([P, P], F32, tag="ones32")
            nc.vector.memset(ones32[:], 1.0)
            onesr = pp.tile([P, P], F32R, tag="onesr")
            nc.vector.tensor_copy(out=onesr[:], in_=ones32[:])

            iotai = pp.tile([P, CAP], I32, tag="iotai")
            nc.gpsimd.iota(iotai[:], pattern=[[1, CAP]], base=1,
                           channel_multiplier=0)
            iotaf = pp.tile([P, CAP], F32, tag="iotaf")
            nc.vector.tensor_copy(out=iotaf[:], in_=iotai[:])

            repc = pp.tile([P, 8], F32, tag="repc")
            nc.vector.memset(repc[:], -1.0)
            toki = pp.tile([P, NT], I32, tag="toki")
            nc.gpsimd.iota(toki[:], pattern=[[P, NT]], base=0,
                           channel_multiplier=1)   # toki[p, m] = m*128 + p
            tokr = pp.tile([P, NT], F32R, tag="tokr")
            nc.vector.tensor_copy(out=tokr[:], in_=toki[:])
            ebase = pp.tile([P, E], F32, tag="ebase")
            for e in range(E):
                nc.vector.memset(ebase[:, e:e + 1], float(e * CAP))

            # ---------------- load xT, router weights ----------------
            rwt = pp.tile([P, KT, E], F32, tag="rwt")
            nc.sync.dma_start(
                out=rwt[:], in_=rwT.rearrange("(kk p) e -> p kk e", p=P))
            xt = [pp.tile([P, T], F32, tag=f"xt{kk}", name=f"xtt{kk}")
                  for kk in range(KT)]
            for m in range(2):
                for kk in range(KT):
                    nc.sync.dma_start(
                        out=xt[kk][:, m * P:(m + 1) * P],
                        in_=xT[kk * P:(kk + 1) * P, m * P:(m + 1) * P])
            for kk in range(KT):
                nc.sync.dma_start(out=xt[kk][:, 2 * P:T],
                                  in_=xT[kk * P:(kk + 1) * P, 2 * P:T])

            # ---------------- router + slot positions ----------------
            # riw columns: 0 token id, 1..8 combine weight per expert
            NW = 1 + E
            # expert-0 inverse-perm accumulates inside the router loop so its
            # gather can fire as soon as routing finishes
            pipsT0 = psp.tile([NW, CAP], F32, tag="pips", name="pip0")
            maskr, qtiles, rhsiw, sidx_ab = [], [], [], []
            for m in range(NT):
                pl = ps2.tile([P, E], F32, tag="ps2", name=f"pl{m}")
                for kk in range(KT):
                    nc.tensor.matmul(
                        pl[:], xt[kk][:, m * P:(m + 1) * P], rwt[:, kk, :],
                        start=(kk == 0), stop=(kk == KT - 1))
                top8l = tp.tile([P, 8], F32, tag="t8l", name="t8l")
                nc.vector.max(out=top8l[:], in_=pl[:])
                negm = tp.tile([P, 1], F32, tag="negm", name="negm")
                nc.vector.tensor_scalar_mul(negm[:], top8l[:, 0:1], -1.0)
                exps = tp.tile([P, E], F32, tag="exps", name="exps")
                sume = tp.tile([P, 1], F32, tag="sume", name="sume")
                nc.scalar.activation(out=exps[:], in_=pl[:], func=Exp,
                                     bias=negm[:, 0:1], accum_out=sume[:, 0:1])
                rz = tp.tile([P, 1], F32, tag="rz", name="rz")
                nc.vector.reciprocal(rz[:], sume[:])
                probs = tp.tile([P, E], F32, tag="probs", name="probs")
                nc.vector.tensor_scalar_mul(probs[:], exps[:], rz[:, 0:1])
                top8p = tp.tile([P, 8], F32, tag="t8p", name="t8p")
                nc.vector.max(out=top8p[:], in_=probs[:])
                den = tp.tile([P, 1], F32, tag="den", name="den")
                nc.vector.tensor_scalar(den[:], top8p[:, 0:1],
                                        top8p[:, 1:2], 1e-6,
                                        Alu.add, Alu.add)
                rden = tp.tile([P, 1], F32, tag="rden", name="rden")
                nc.vector.reciprocal(rden[:], den[:])
                repin = tp.tile([P, 8], F32, tag="repin", name="repin")
                nc.vector.tensor_copy(out=repin[:, 2:8], in_=repc[:, 2:8])
                nc.vector.tensor_copy(out=repin[:, 0:2], in_=top8p[:, 0:2])
                repl = tp.tile([P, 8], F32, tag="repl", name="repl")
                nc.vector.match_replace(out=repl[:], in_to_replace=repin[:],
                                        in_values=probs[:], imm_value=-2.0)
                mask = tp.tile([P, E], F32, tag="maskt", name="maskt")
                nc.vector.tensor_tensor(out=mask[:], in0=probs[:], in1=repl[:],
                                        op=Alu.not_equal)
                mr = pp.tile([P, E], F32R, tag=f"maskr{m}", name=f"maskr{m}")
                nc.vector.tensor_copy(out=mr[:], in_=mask[:])
                maskr.append(mr)
                cw = tp.tile([P, E], F32, tag="cw", name="cw")
                nc.vector.tensor_tensor(out=cw[:], in0=probs[:], in1=mask[:],
                                        op=Alu.mult)
                nc.vector.tensor_scalar_mul(cw[:], cw[:], rden[:, 0:1])

                ppos = ps2.tile([P, E], F32, tag="ps2", name=f"ppos{m}")
                if m == 0:
                    nc.tensor.matmul(ppos[:], trir[:], maskr[0][:],
                                     start=True, stop=True)
                else:
                    for mp in range(m):
                        nc.tensor.matmul(ppos[:], onesr[:], maskr[mp][:],
                                         start=(mp == 0), stop=False)
                    nc.tensor.matmul(ppos[:], trir[:], maskr[m][:],
                                     start=False, stop=True)
                q = pp.tile([P, E], F32, tag=f"q{m}", name=f"q{m}")
                nc.vector.tensor_tensor(out=q[:], in0=ppos[:], in1=mask[:],
                                        op=Alu.mult)
                qtiles.append(q)

                riw = pp.tile([P, NW], F32R, tag=f"riw{m}", name=f"riw{m}")
                nc.vector.tensor_copy(out=riw[:, 0:1], in_=tokr[:, m:m + 1])
                nc.vector.tensor_copy(out=riw[:, 1:1 + E], in_=cw[:])
                rhsiw.append(riw)

                it0 = tp.tile([P, CAP], F32R, tag="ieq0", name="ieq0")
                nc.vector.tensor_tensor(
                    out=it0[:],
                    in0=q[:, 0:1].to_broadcast([P, CAP]),
                    in1=iotaf[:], op=Alu.is_equal)
                nc.tensor.matmul(pipsT0[:], riw[:], it0[:],
                                 start=(m == 0), stop=(m == NT - 1))

                # global slot index per (t, e); BIG where not selected
                slotg = tp.tile([P, E], F32, tag="slotg", name="slotg")
                nc.vector.tensor_tensor(out=slotg[:], in0=q[:], in1=ebase[:],
                                        op=Alu.add)
                nc.vector.tensor_scalar_add(slotg[:], slotg[:], -1.0)
                slotm = tp.tile([P, E], F32, tag="slotm", name="slotm")
                nc.vector.tensor_scalar_add(slotm[:], slotg[:], -BIG)
                nc.vector.tensor_tensor(out=slotm[:], in0=slotm[:],
                                        in1=mask[:], op=Alu.mult)
                nc.vector.tensor_scalar_add(slotm[:], slotm[:], BIG)
                negs = tp.tile([P, E], F32, tag="negs", name="negs")
                nc.vector.tensor_scalar_mul(negs[:], slotm[:], -1.0)
                mn8 = tp.tile([P, 8], F32, tag="mn8", name="mn8")
                nc.vector.max(out=mn8[:], in_=negs[:])
                sab = pp.tile([P, 2], I32, tag=f"sab{m}", name=f"sab{m}")
                nc.vector.tensor_scalar_mul(sab[:], mn8[:, 0:2], -1.0)
                sidx_ab.append(sab)

            # ---------------- inverse permutation per expert ----------------
            sidx = [[None] * ST for _ in range(E)]
            swt = [[None] * ST for _ in range(E)]

            def perm_extract(e, pipsT):
                # pipsT is [NW, CAP] psum; transpose each slot tile back to
                # (slots, NW) and pull out index / weight columns.
                pts = tp.tile([NW, CAP], F32, tag="ptsb", name=f"ptsb{e}")
                nc.vector.tensor_copy(out=pts[:], in_=pipsT[:])
                for st in range(ST):
                    sz = SZ[st]
                    tps = ptr.tile([P, NW], F32, tag="ptr",
                                   name=f"tps{e}_{st}")
                    nc.tensor.transpose(
                        out=tps[:sz, :NW],
                        in_=pts[:, SOFF[st]:SOFF[st] + sz],
                        identity=ident32[:NW, :NW])
                    si = pp.tile([SZ[st], 1], I32, tag=f"si{e}_{st}",
                                 name=f"si{e}_{st}")
                    nc.vector.tensor_copy(out=si[:], in_=tps[:sz, 0:1])
                    sw = pp.tile([SZ[st], 1], F32, tag=f"sw{e}_{st}",
                                 name=f"sw{e}_{st}")
                    nc.vector.tensor_copy(out=sw[:],
                                          in_=tps[:sz, 1 + e:2 + e])
                    sidx[e][st] = si
                    swt[e][st] = sw

            perm_extract(0, pipsT0)

            def perm_eq(e):
                its = []
                for m in range(NT):
                    it = eqp.tile([P, CAP], F32R, tag=f"ieq{m}",
                                  name=f"ieq{e}_{m}")
                    nc.vector.tensor_tensor(
                        out=it[:],
                        in0=qtiles[m][:, e:e + 1].to_broadcast([P, CAP]),
                        in1=iotaf[:],
                        op=Alu.is_equal)
                    its.append(it)
                return its

            def perm_block(e, its):
                pipsT = psp.tile([NW, CAP], F32, tag="pips", name=f"pip{e}")
                for m in range(NT):
                    nc.tensor.matmul(pipsT[:], rhsiw[m][:], its[m][:],
                                     start=(m == 0), stop=(m == NT - 1))
                perm_extract(e, pipsT)

            # ---------------- per-expert compute (sw-pipelined) ----------
            hsb = [None] * 16

            def gather_and_transpose(e):
                xgt = [xtp.tile([P, CAP], BF16, tag=f"xgt{kk}",
                                name=f"xgt{kk}_{e}") for kk in range(KT)]
                for st in range(ST):
                    sz = SZ[st]
                    xg = xgp.tile([P, H], BF16, tag="xg", name=f"xg{e}_{st}")
                    nc.gpsimd.indirect_dma_start(
                        out=xg[:sz, :], out_offset=None,
                        in_=xrow[:],
                        in_offset=bass.IndirectOffsetOnAxis(
                            ap=sidx[e][st][:, 0:1], axis=0))
                    for kk in range(KT):
                        pt = ptr.tile([P, P], BF16, tag="ptr",
                                      name=f"pt{e}_{st}_{kk}")
                        nc.tensor.transpose(
                            out=pt[:P, :sz], in_=xg[:sz, kk * P:(kk + 1) * P],
                            identity=identb[:sz, :sz])
                        nc.vector.tensor_copy(
                            out=xgt[kk][:, SOFF[st]:SOFF[st] + sz],
                            in_=pt[:P, :sz])
                return xgt

            xgt_next = gather_and_transpose(0)
            its_next = perm_eq(1)
            for e in range(E):
                xgt = xgt_next
                # kick off expert e+1's perm + gather chain first so the
                # indirect DMA lands while expert e's GEMMs occupy the PE
                if e + 1 < E:
                    perm_block(e + 1, its_next)
                    xgt_next = gather_and_transpose(e + 1)
                    if e + 2 < E:
                        its_next = perm_eq(e + 2)

                # GEMM1 (bf16) + SwiGLU -> h (bf16), transposed (I, slots)
                w13r = w13[e].rearrange("(kk p) i -> p kk i", p=P)
                for c in range(8):
                    wt = wp1.tile([P, KT, 512], BF16, tag="w13t",
                                  name=f"w13t{e}_{c}")
                    nc.sync.dma_start(
                        out=wt[:], in_=w13r[:, :, c * 512:(c + 1) * 512])
                    for j in range(4):
                        g = c * 4 + j
                        pg = ps1.tile([P, CAP], F32, tag="ps1",
                                      name=f"pg{e}_{g}")
                        for kk in range(KT):
                            nc.tensor.matmul(
                                pg[:], wt[:, kk, j * P:(j + 1) * P],
                                xgt[kk][:],
                                start=(kk == 0), stop=(kk == KT - 1))
                        if g < 16:
                            ht = hp.tile([P, CAP], BF16, tag=f"h{g}",
                                         name=f"h{g}_{e}")
                            hsb[g] = ht
                            nc.scalar.activation(out=ht[:], in_=pg[:],
                                                 func=Silu)
                        else:
                            nc.vector.tensor_tensor(
                                out=hsb[g - 16][:], in0=hsb[g - 16][:],
                                in1=pg[:], op=Alu.mult)

                # GEMM2 (bf16): yT[h-tile, slots] = w2[e]^T-blocks @ h, so the
                # matmul free dim is the slot count; transpose back per slot
                # tile with per-slot routing-weight scaling on the psum read.
                ysb = [yp.tile([SZ[st], H], BF16, tag=f"ysb{st}",
                               name=f"ysb{e}_{st}") for st in range(ST)]
                for h in range(KT):
                    w2ct = wp2.tile([P, I // P, P], BF16, tag="w2t",
                                    name=f"w2t{e}_{h}")
                    nc.sync.dma_start(out=w2ct[:], in_=w2c[e, h])
                    pyt = ps2.tile([P, CAP], F32, tag="ps2",
                                   name=f"pyt{e}_{h}")
                    for kk2 in range(16):
                        nc.tensor.matmul(pyt[:], w2ct[:, kk2, :],
                                         hsb[kk2][:],
                                         start=(kk2 == 0), stop=(kk2 == 15))
                    ytb = tp.tile([P, CAP], BF16, tag="ytb",
                                  name=f"ytb{e}_{h}")
                    nc.vector.tensor_copy(out=ytb[:], in_=pyt[:])
                    for st in range(ST):
                        sz = SZ[st]
                        ptt = ptr.tile([P, P], BF16, tag="ptr",
                                       name=f"yt{e}_{h}_{st}")
                        nc.tensor.transpose(
                            out=ptt[:sz, :P],
                            in_=ytb[:, SOFF[st]:SOFF[st] + sz],
                            identity=identb[:, :])
                        nc.scalar.activation(
                            out=ysb[st][:, h * P:(h + 1) * P],
                            in_=ptt[:sz, :P], func=Copy,
                            scale=swt[e][st][:, 0:1])
                for st in range(ST):
                    nc.sync.dma_start(
                        out=yslots[e * CAP + SOFF[st]:
                                   e * CAP + SOFF[st] + SZ[st], :],
                        in_=ysb[st][:])

            # ---------------- final combine ----------------
            for m in range(NT):
                sab = sidx_ab[m]
                ga = tp.tile([P, H], BF16, tag="ga", name=f"ga{m}")
                nc.gpsimd.indirect_dma_start(
                    out=ga[:], out_offset=None, in_=yslots[:],
                    in_offset=bass.IndirectOffsetOnAxis(ap=sab[:, 0:1],
                                                        axis=0))
                gb = tp.tile([P, H], BF16, tag="gb", name=f"gb{m}")
                nc.gpsimd.indirect_dma_start(
                    out=gb[:], out_offset=None, in_=yslots[:],
                    in_offset=bass.IndirectOffsetOnAxis(ap=sab[:, 1:2],
                                                        axis=0))
                go = tp.tile([P, H], BF16, tag="go", name=f"go{m}")
                nc.vector.tensor_tensor(out=go[:], in0=ga[:], in1=gb[:],
                                        op=Alu.add)
                nc.sync.dma_start(out=out[m * P:(m + 1) * P, :], in_=go[:])

    nc.compile()
    return nc


_prog = None


def _balanced_token_perm(xrows, router_w):
    """Assign tokens to cores so per-(core, expert) routed counts stay
    well under CAP (global max expert load / 8 is ~271).  Routing here is
    the same fp32 math the device performs; the min top2/top3 probability
    gap in this data (~2e-5) is far above fp32 noise, so host and device
    agree on the selected experts."""
    logits = (xrows @ router_w.T).astype(np.float32)
    m = logits.max(-1, keepdims=True)
    p = np.exp(logits - m)
    p /= p.sum(-1, keepdims=True)
    idx = np.argsort(-p, axis=-1)[:, :2]
    N = xrows.shape[0]
    counts = np.zeros((NCORES, E), dtype=np.int64)
    sizes = np.zeros(NCORES, dtype=np.int64)
    asgn = np.empty(N, dtype=np.int64)
    for t in range(N):
        e1, e2 = idx[t]
        best, bkey = -1, None
        for c in range(NCORES):
            if sizes[c] >= T:
                continue
            key = (max(counts[c, e1], counts[c, e2]),
                   counts[c, e1] + counts[c, e2], sizes[c])
            if bkey is None or key < bkey:
                bkey, best = key, c
        asgn[t] = best
        counts[best, e1] += 1
        counts[best, e2] += 1
        sizes[best] += 1
    assert counts.max() <= CAP - 4, f"capacity overflow risk: {counts.max()}"
    return np.argsort(asgn, kind="stable")


def kernel(x, router_w, w13, w2):
    global _prog, LAST_RESULTS
    if _prog is None:
        _prog = _build_program()
    nc = _prog

    xrows = x.reshape(NCORES * T, H).astype(np.float32)
    perm = _balanced_token_perm(xrows, np.asarray(router_w, np.float32))
    xrows = np.ascontiguousarray(xrows[perm])
    xt_full = np.ascontiguousarray(xrows.T)
    rwT_np = np.ascontiguousarray(router_w.T).astype(np.float32)
    w13_b = np.ascontiguousarray(w13).astype(ml_dtypes.bfloat16)
    # w2c[e, h, p, kk2, c] = w2[e, kk2*128+p, h*128+c]
    w2_b = np.ascontiguousarray(
        np.asarray(w2).reshape(E, I // 128, 128, H // 128, 128)
        .transpose(0, 3, 2, 1, 4)).astype(ml_dtypes.bfloat16)

    in_maps = []
    for c in range(NCORES):
        in_maps.append({
            "xT": np.ascontiguousarray(xt_full[:, c * T:(c + 1) * T]),
            "xrow": np.ascontiguousarray(
                xrows[c * T:(c + 1) * T]).astype(ml_dtypes.bfloat16),
            "rwT": rwT_np,
            "w13": w13_b,
            "w2c": w2_b,
        })

    res = run_bass_kernel_spmd(nc, in_maps, core_ids=list(range(NCORES)))
    LAST_RESULTS = res
    outs = [np.asarray(res.results[c]["out"]).astype(np.float32)
            for c in range(NCORES)]
    full = np.concatenate(outs, axis=0)
    unperm = np.empty_like(full)
    unperm[perm] = full
    return unperm.reshape(4, 2048, H).astype(x.dtype, copy=False)

